# revision 29
# baseline (speedup 1.0000x reference)
"""Multi-head attention (B=4, S=2048, E=768, H=12) on 8 trn2 NeuronCores.

Sharding: 2-D (batch x head-half). Core c handles batch c//2, heads
(c%2)*6 .. (c%2)*6+5  (Wq/Wk/Wv column-split, Wo row-split). Each core
returns a partial O^T [768, S]; host sums the two head-halves per batch,
transposes, and adds the effective output bias (bo + bv@Wo — softmax rows
sum to 1, so V's bias contributes a constant row folded on the host).

Device kernel (per core), bf16 matmuls + fp32 PSUM:
  - masked keys are compacted away on host; padded keys get -30000 added
    via the exp's per-partition bias -> exp == 0.
  - scores/ctx computed transposed (S^T tiles [128 k, q]) so P^T feeds the
    context matmul directly; V carries an appended ones column so row 64
    of the context accumulator is the softmax denominator.
  - the scalar (ACT) engine runs ONLY the exps; each head-pair's two score
    tiles live in one [128, 2, 512] PSUM tile so a single exp covers both.
  - normalization per (head, qblock): DMA the denominator row to
    partition 0, reciprocal there, exact hi/lo bf16 split, and broadcast
    across the 64 context rows with two accumulating K=1 ones-matmuls.
  - inputs stream in deadline order (K, Q-head, V-head, V-tail, Q-tail);
    only K/Q m=0 head-tiles project before attention — everything else
    (V, remaining projections, O-projection, normalization) is deferred
    into the attention stream in ~1us chunks popped once per key-chunk,
    so exps start ~15us in and the PE stays dense (HAM at full clock).
    Attention blocks run head-pair-outer so deferred m=1,2 projections
    have 4 blocks of slack before their consumers.
"""

import os
import numpy as np
import ml_dtypes

E = 768
H = 12
D = 64
HALF = 384  # E // 2 output cols per head-half
N_CORES = 8
QB = 512    # query block

_CACHE = {}
_LAST = None  # last BassKernelResults (for test harness introspection)

bf16_np = ml_dtypes.bfloat16


def _build(S_q, S_pad):
    from contextlib import ExitStack
    import concourse.bass as bass
    import concourse.tile as tile
    from concourse import bacc, mybir

    bf16 = mybir.dt.bfloat16
    f32 = mybir.dt.float32
    FT = mybir.ActivationFunctionType

    NKC = S_pad // 128
    NMC = HALF // 128        # 3 proj-dim chunks (head pairs)
    NEC = E // 128           # 6 embed chunks
    NQB = S_q // QB          # query blocks
    QA = 1024                # qT head split (first two n-tiles)
    VA = 512                 # vT head split (first four key chunks)

    def ntiles(total, step=512):
        return [(s, min(step, total - s)) for s in range(0, total, step)]

    nc = bacc.Bacc("TRN2", target_bir_lowering=False, debug=False,
                   num_devices=N_CORES)

    # inputs are interleaved host-side to [128, chunks, cols] so each
    # tensor loads with a single large DMA (trigger rate, not bandwidth,
    # limits the input stream otherwise)
    qTa = nc.dram_tensor("qTa", [128, NEC, QA], bf16, kind="ExternalInput").ap()
    qTb = nc.dram_tensor("qTb", [128, NEC, S_q - QA], bf16, kind="ExternalInput").ap()
    KA = 1024                # kT head split (first two n-tiles)
    kTa = nc.dram_tensor("kTa", [128, NEC, KA], bf16, kind="ExternalInput").ap()
    kTb = nc.dram_tensor("kTb", [128, NEC, S_pad - KA], bf16, kind="ExternalInput").ap()
    vTa = nc.dram_tensor("vTa", [128, NEC, VA], bf16, kind="ExternalInput").ap()
    vTb = nc.dram_tensor("vTb", [128, NEC, S_pad - VA], bf16, kind="ExternalInput").ap()
    wq = nc.dram_tensor("wq", [128, NEC, HALF], bf16, kind="ExternalInput").ap()
    wk = nc.dram_tensor("wk", [128, NEC, HALF], bf16, kind="ExternalInput").ap()
    wv = nc.dram_tensor("wv", [128, NEC, HALF], bf16, kind="ExternalInput").ap()
    wo = nc.dram_tensor("wo", [128, NMC, E], bf16, kind="ExternalInput").ap()
    bq2 = nc.dram_tensor("bq2", [128, NMC], f32, kind="ExternalInput").ap()
    bk2 = nc.dram_tensor("bk2", [128, NMC], f32, kind="ExternalInput").ap()
    kbias = nc.dram_tensor("kbias", [128, NKC], f32, kind="ExternalInput").ap()
    oT = nc.dram_tensor("oT", [E, S_q], bf16, kind="ExternalOutput").ap()

    with tile.TileContext(nc) as tc, ExitStack() as ctx:
        cons = ctx.enter_context(tc.tile_pool(name="cons", bufs=1))
        wp = ctx.enter_context(tc.tile_pool(name="wp", bufs=1))
        acts = ctx.enter_context(tc.tile_pool(name="acts", bufs=1))
        pp = ctx.enter_context(tc.tile_pool(name="pp", bufs=12))
        ost = ctx.enter_context(tc.tile_pool(name="ost", bufs=4))
        nrm = ctx.enter_context(tc.tile_pool(name="nrm", bufs=1))

        # ---- constant/small loads ----
        bq2_t = cons.tile([128, NMC], f32, tag="bq2")
        bk2_t = cons.tile([128, NMC], f32, tag="bk2")
        kb_t = cons.tile([128, NKC], f32, tag="kb")
        ones2 = cons.tile([2, 64], bf16, tag="ones2")
        nc.sync.dma_start(bq2_t[:], bq2[:])
        nc.sync.dma_start(bk2_t[:], bk2[:])
        nc.sync.dma_start(kb_t[:], kbias[:])
        nc.vector.memset(ones2[:], 1.0)

        # ---- input loads in deadline order, two trigger queues ----
        qkv = tc.tile_pool(name="qkv", bufs=1)
        inp = qkv.__enter__()
        wq_t = wp.tile([128, NEC, HALF], bf16, tag="wq", name="wq")
        wk_t = wp.tile([128, NEC, HALF], bf16, tag="wk", name="wk")
        wv_t = wp.tile([128, NEC, HALF], bf16, tag="wv", name="wv")
        wo_t = wp.tile([128, NMC, E], bf16, tag="wo", name="wo")
        kTa_t = inp.tile([128, NEC, KA], bf16, tag="kTa", name="kTat")
        kTb_t = inp.tile([128, NEC, S_pad - KA], bf16, tag="kTb", name="kTbt")
        qTa_t = inp.tile([128, NEC, QA], bf16, tag="qTa", name="qTat")
        qTb_t = inp.tile([128, NEC, S_q - QA], bf16, tag="qTb", name="qTbt")
        vTa_t = inp.tile([128, NEC, VA], bf16, tag="vTa", name="vTat")
        vTb_t = inp.tile([128, NEC, S_pad - VA], bf16, tag="vTb", name="vTbt")
        nc.sync.dma_start(wk_t[:], wk[:])
        nc.gpsimd.dma_start(wq_t[:], wq[:])
        nc.sync.dma_start(kTa_t[:], kTa[:])
        nc.gpsimd.dma_start(qTa_t[:], qTa[:])
        nc.sync.dma_start(wv_t[:], wv[:])
        nc.sync.dma_start(vTa_t[:], vTa[:])
        nc.gpsimd.dma_start(vTb_t[:], vTb[:])
        nc.sync.dma_start(kTb_t[:], kTb[:])
        nc.gpsimd.dma_start(qTb_t[:], qTb[:])
        nc.sync.dma_start(wo_t[:], wo[:])

        def wqx(e):
            return wq_t[:, e, :]

        def wkx(e):
            return wk_t[:, e, :]

        def qx(e, n0, nw):
            if n0 + nw <= QA:
                return qTa_t[:, e, n0:n0 + nw]
            return qTb_t[:, e, n0 - QA:n0 - QA + nw]

        def kx(e, n0, nw):
            if n0 + nw <= KA:
                return kTa_t[:, e, n0:n0 + nw]
            return kTb_t[:, e, n0 - KA:n0 - KA + nw]

        def vx(e, j):  # key chunk j of vT
            if 128 * (j + 1) <= VA:
                return vTa_t[:, e, 128 * j:128 * (j + 1)]
            return vTb_t[:, e, 128 * j - VA:128 * (j + 1) - VA]

        kts = [acts.tile([128, S_pad], bf16, tag=f"kts{m}", name=f"kts{m}") for m in range(NMC)]
        qts = [acts.tile([128, S_q], bf16, tag=f"qts{m}", name=f"qts{m}") for m in range(NMC)]
        vhx = [acts.tile([128, 6, 128], bf16, tag=f"vhx{j}", name=f"vhx{j}") for j in range(NKC)]
        czT = [acts.tile([128, S_q], bf16, tag=f"czT{m}", name=f"czT{m}") for m in range(NMC)]

        def proj_mm(ps_, tagf, wt, xf, m, pair, e_lo, e_hi, pjs):
            if not pjs:
                pjs.extend(ps_.tile([128, 512], f32, tag=tagf(i), bufs=2,
                                    name=f"pj{m}_{pair[i][0]}_{id(wt)}")
                           for i in range(len(pair)))
            for e in range(e_lo, e_hi):
                for i, (n0, nw) in enumerate(pair):
                    nc.tensor.matmul(pjs[i][:, :nw],
                                     wt[:, e, 128 * m:128 * (m + 1)],
                                     xf(e, n0, nw),
                                     start=(e == 0), stop=(e == NEC - 1))

        def proj_evac(out, bias_t, m, pair, pjs):
            for i, (n0, nw) in enumerate(pair):
                nc.vector.tensor_scalar_add(out[m][:, n0:n0 + nw],
                                            pjs[i][:, :nw],
                                            bias_t[:, m:m + 1])

        def v_mm(ps_, tagf, chunks, e_lo, e_hi, pvs):
            if not pvs:
                pvs.extend(ps_.tile([128, 512], f32, tag=tagf(i), bufs=2,
                                    name=f"pv{chunks[i]}")
                           for i in range(len(chunks)))
            for e in range(e_lo, e_hi):
                for i, j in enumerate(chunks):
                    nc.tensor.matmul(pvs[i][:, 0:HALF], vx(e, j),
                                     wv_t[:, e, :],
                                     start=(e == 0), stop=(e == NEC - 1))

        def v_evac(chunks, pvs):
            for i, j in enumerate(chunks):
                nc.vector.memset(vhx[j][:, :, 64:128], 1.0)
                nc.vector.tensor_copy(
                    vhx[j][:, :, 0:64],
                    pvs[i][:, 0:HALF].rearrange("p (h d) -> p h d", h=6))

        # ---- prologue: K m0 n0-1 then Q m0 n0-1 (from the early tiles) ----
        kt2 = ntiles(S_pad)
        qt2 = ntiles(S_q)
        psp = tc.tile_pool(name="psp", bufs=1, space="PSUM")
        ps = psp.__enter__()
        pjs = []
        proj_mm(ps, lambda i: f"pj{i}", wk_t, kx, 0, kt2[0:2], 0, NEC, pjs)
        proj_evac(kts, bk2_t, 0, kt2[0:2], pjs)
        pjs = []
        proj_mm(ps, lambda i: f"pj{2 + i}", wq_t, qx, 0, qt2[0:2], 0, NEC, pjs)
        proj_evac(qts, bq2_t, 0, qt2[0:2], pjs)
        psp.__exit__(None, None, None)

        # ---- attention ----
        psa = tc.tile_pool(name="psa", bufs=1, space="PSUM")
        ps = psa.__enter__()

        deferred = []   # single FIFO: pops can never interleave an open
                        # accumulation because pushes only append
        pend_b = []     # normalization B-stages awaiting a 1-block lag
        hl_t = {}
        pushed_units = [0] * NQB

        def defer_proj(wt, xf, out, bias_t, m, pair, halves=True):
            pjs = []
            mid = NEC // 2 if halves else NEC

            def h1():
                proj_mm(ps, lambda i: "aux", wt, xf, m, pair, 0, mid, pjs)
                if not halves:
                    proj_evac(out, bias_t, m, pair, pjs)

            def h2():
                proj_mm(ps, lambda i: "aux", wt, xf, m, pair, mid, NEC, pjs)
                proj_evac(out, bias_t, m, pair, pjs)
            deferred.extend([h1, h2] if halves else [h1])

        def defer_v(chunks):
            pvs = []

            def h1():
                v_mm(ps, lambda i: "aux", chunks, 0, NEC // 2, pvs)

            def h2():
                v_mm(ps, lambda i: "aux", chunks, NEC // 2, NEC, pvs)
                v_evac(chunks, pvs)
            deferred.extend([h1, h2])

        # queue order == deadline order; see module docstring.
        for c0 in range(0, NKC, 2):
            defer_v(list(range(c0, min(c0 + 2, NKC))))
        if kt2[2:]:
            defer_proj(wk_t, kx, kts, bk2_t, 0, kt2[2:], halves=False)
        if qt2[2:4]:
            defer_proj(wq_t, qx, qts, bq2_t, 0, qt2[2:4])
        for m in range(1, NMC):
            defer_proj(wk_t, kx, kts, bk2_t, m, kt2[0:2])
            if kt2[2:]:
                defer_proj(wk_t, kx, kts, bk2_t, m, kt2[2:], halves=False)
            defer_proj(wq_t, qx, qts, bq2_t, m, qt2[0:2])
            if qt2[2:4]:
                defer_proj(wq_t, qx, qts, bq2_t, m, qt2[2:4])

        def make_unit_a(u, dn):
            def unit_a():
                # r = 1/den at partition 0; exact bf16 split r = hi + lo.
                rq = nrm.tile([1, QB], f32, tag="rq", bufs=4, name=f"rq{u}")
                nc.vector.reciprocal_approx_fast(rq[:], dn[:])
                hl = nrm.tile([1, QB], bf16, tag="hl", bufs=4, name=f"hl{u}")
                nc.vector.tensor_copy(hl[:], rq[:])
                lo = nrm.tile([1, QB], bf16, tag="lo", bufs=4, name=f"lo{u}")
                nc.vector.tensor_sub(lo[:], rq[:], hl[:])
                hl_t[u] = (hl, lo)
            return unit_a

        def make_unit_b(u, cs):
            qb, h = divmod(u, 6)
            m, half = divmod(h, 2)

            def unit_b():
                hl, lo = hl_t.pop(u)
                bcp = ps.tile([128, QB], f32, tag="aux", bufs=2,
                              name=f"bcp{u}")
                nc.tensor.matmul(bcp[0:64, :], ones2[0:1, :], hl[:],
                                 start=True, stop=False)
                nc.tensor.matmul(bcp[0:64, :], ones2[0:1, :], lo[:],
                                 start=False, stop=True)
                nc.vector.tensor_mul(
                    czT[m][64 * half:64 * (half + 1),
                           qb * QB:(qb + 1) * QB],
                    cs[0:64, :], bcp[0:64, :])
            return unit_b

        def make_oproj(qb):
            fns = []
            for e0 in range(0, NEC, 2):
                def fn(e0=e0, qb=qb):
                    ecs = (e0, e0 + 1)
                    pos = [ps.tile([128, QB], f32, tag="aux", bufs=2,
                                   name=f"po{qb}_{ec}") for ec in ecs]
                    for mm in range(NMC):
                        for j, ec in enumerate(ecs):
                            nc.tensor.matmul(
                                pos[j][:],
                                wo_t[:, mm, 128 * ec:128 * (ec + 1)],
                                czT[mm][:, qb * QB:(qb + 1) * QB],
                                start=(mm == 0), stop=(mm == NMC - 1))
                    for j, ec in enumerate(ecs):
                        ot = ost.tile([128, QB], bf16, tag="ot",
                                      name=f"ot{qb}_{ec}")
                        nc.vector.tensor_copy(ot[:], pos[j][:])
                        nc.sync.dma_start(
                            oT[128 * ec:128 * (ec + 1),
                               qb * QB:(qb + 1) * QB], ot[:])
                fns.append(fn)
            return fns

        def evac(u, C):
            # context + denominator row out of PSUM; den row DMA'd to
            # partition 0 for the A-stage reciprocal.
            cs = nrm.tile([65, QB], f32, tag="cs", bufs=8, name=f"cs{u}")
            nc.vector.tensor_copy(cs[:], C[0:65, :])
            dn = nrm.tile([1, QB], f32, tag="dn", bufs=4, name=f"dn{u}")
            nc.sync.dma_start(dn[:], cs[64:65, :])
            deferred.append(make_unit_a(u, dn))
            pend_b.append((u, make_unit_b(u, cs)))

        def flush_b():
            # push last block's B-stages (their hi/lo is ready by now) and
            # any O-projection whose 6 units are all queued.
            while pend_b:
                u, fn = pend_b.pop(0)
                deferred.append(fn)
                qb = u // 6
                pushed_units[qb] += 1
                if pushed_units[qb] == 6:
                    deferred.extend(make_oproj(qb))

        pend_cs = []  # [(u, C), ...] awaiting evacuation
        nblk = 0
        for p in range(NMC):      # head pair: hA=2p (rows 0-63), hB=2p+1
            for qb in range(NQB):
                q0 = qb * QB
                hA, hB = 2 * p, 2 * p + 1
                CA = ps.tile([128, QB], f32, tag="CA", name=f"CA{qb}_{p}")
                CB = ps.tile([128, QB], f32, tag="CB", name=f"CB{qb}_{p}")
                flush_b()
                for pc in pend_cs:
                    evac(*pc)
                pend_cs = []

                def sc_pair(kc, St):
                    nc.tensor.matmul(
                        St[:, 0, :], kts[p][0:64, 128 * kc:128 * (kc + 1)],
                        qts[p][0:64, q0:q0 + QB],
                        start=True, stop=True, tile_position=(0, 0))
                    nc.tensor.matmul(
                        St[:, 1, :], kts[p][64:128, 128 * kc:128 * (kc + 1)],
                        qts[p][64:128, q0:q0 + QB],
                        start=True, stop=True, tile_position=(64, 0))

                S = ps.tile([128, 2, QB], f32, tag="S", bufs=2,
                            name=f"S{qb}_{p}_0")
                sc_pair(0, S)
                for kc in range(NKC):
                    # block 0 pops from kc0 and doubled: the V-projection
                    # fns must all EMIT before their first in-block readers
                    # (vhx[j] is read by the kc=j context matmul).
                    if (kc >= 1 or nblk == 0) and deferred:
                        deferred.pop(0)()
                        if ((nblk == 0 or (nblk >= 2 and len(deferred) > 20))
                                and deferred):
                            deferred.pop(0)()
                    S2 = None
                    if kc + 1 < NKC:
                        S2 = ps.tile([128, 2, QB], f32, tag="S", bufs=2,
                                     name=f"S{qb}_{p}_{kc + 1}")
                        sc_pair(kc + 1, S2)
                    if kc == 5:
                        flush_b()
                    P = pp.tile([128, 2, QB], bf16, tag="P",
                                name=f"P{qb}_{p}_{kc}")
                    nc.scalar.activation(P[:], S[:], FT.Exp,
                                         bias=kb_t[:, kc:kc + 1], scale=1.0)
                    nc.tensor.matmul(CA[:], vhx[kc][:, hA, :], P[:, 0, :],
                                     start=(kc == 0), stop=(kc == NKC - 1))
                    nc.tensor.matmul(CB[:], vhx[kc][:, hB, :], P[:, 1, :],
                                     start=(kc == 0), stop=(kc == NKC - 1))
                    S = S2
                pend_cs = [(qb * 6 + hA, CA), (qb * 6 + hB, CB)]
                nblk += 1

        # flush: evacuate last heads, push their B-stages, drain the queue
        flush_b()
        for pc in pend_cs:
            evac(*pc)
        flush_b()
        while deferred:
            deferred.pop(0)()
        psa.__exit__(None, None, None)
        qkv.__exit__(None, None, None)

    nc.compile()
    return nc


def _numpy_fallback(q, k, v, mask, Wq, bq, Wk, bk, Wv, bv, Wo, bo):
    B, Sq, _ = q.shape
    qh = (q @ Wq + bq).reshape(B, Sq, H, D).transpose(0, 2, 1, 3)
    kh = (k @ Wk + bk).reshape(B, -1, H, D).transpose(0, 2, 1, 3)
    vh = (v @ Wv + bv).reshape(B, -1, H, D).transpose(0, 2, 1, 3)
    s = np.einsum("bhqd,bhkd->bhqk", qh, kh) / np.sqrt(np.float32(D))
    s = s + np.where(mask == 0, np.float32(-1e9), np.float32(0))[:, None, None, :]
    s = s - s.max(-1, keepdims=True)
    w = np.exp(s)
    w = w / w.sum(-1, keepdims=True)
    ctx = np.einsum("bhqk,bhkd->bqhd", w, vh).reshape(B, Sq, E)
    return (ctx @ Wo + bo).astype(np.float32)


def kernel(q, k, v, mask, Wq, bq, Wk, bk, Wv, bv, Wo, bo):
    global _LAST
    q = np.asarray(q, np.float32)
    k = np.asarray(k, np.float32)
    v = np.asarray(v, np.float32)
    mask = np.asarray(mask)
    Wq = np.asarray(Wq, np.float32)
    bq = np.asarray(bq, np.float32)
    Wk = np.asarray(Wk, np.float32)
    bk = np.asarray(bk, np.float32)
    Wv = np.asarray(Wv, np.float32)
    bv = np.asarray(bv, np.float32)
    Wo = np.asarray(Wo, np.float32)
    bo = np.asarray(bo, np.float32)

    B, S_q, _ = q.shape
    idxs = [np.flatnonzero(mask[b]) for b in range(B)]
    ns = [len(ix) for ix in idxs]
    if (min(ns) == 0 or B * 2 != N_CORES or S_q % QB != 0
            or S_q < 2048 or max(ns) <= 512):
        return _numpy_fallback(q, k, v, mask, Wq, bq, Wk, bk, Wv, bv, Wo, bo)

    S_pad = ((max(ns) + 127) // 128) * 128
    NKC = S_pad // 128
    NMC = HALF // 128

    key = (S_q, S_pad)
    if key not in _CACHE:
        _CACHE[key] = _build(S_q, S_pad)
    nc = _CACHE[key]

    scale = np.float32(1.0 / np.sqrt(D))

    def ileave(a):  # [n*128, cols] -> [128, n, cols]
        n = a.shape[0] // 128
        return np.ascontiguousarray(
            a.reshape(n, 128, a.shape[1]).transpose(1, 0, 2))

    in_maps = []
    for c in range(N_CORES):
        b, j = divmod(c, 2)
        cols = slice(j * HALF, (j + 1) * HALF)
        kc_ = np.zeros((S_pad, E), np.float32)
        kc_[:ns[b]] = k[b][idxs[b]]
        vc_ = np.zeros((S_pad, E), np.float32)
        vc_[:ns[b]] = v[b][idxs[b]]
        kb_vec = np.zeros(S_pad, np.float32)
        kb_vec[ns[b]:] = -30000.0
        qT_ = np.ascontiguousarray(q[b].T).astype(bf16_np)
        vT_ = np.ascontiguousarray(vc_.T).astype(bf16_np)
        in_maps.append({
            "qTa": ileave(qT_[:, 0:1024]),
            "qTb": ileave(qT_[:, 1024:]),
            "kTa": ileave(np.ascontiguousarray(kc_.T[:, 0:1024]).astype(bf16_np)),
            "kTb": ileave(np.ascontiguousarray(kc_.T[:, 1024:]).astype(bf16_np)),
            "vTa": ileave(vT_[:, 0:512]),
            "vTb": ileave(vT_[:, 512:]),
            "wq": ileave((Wq[:, cols] * scale).astype(bf16_np)),
            "wk": ileave(Wk[:, cols].astype(bf16_np)),
            "wv": ileave(Wv[:, cols].astype(bf16_np)),
            "wo": ileave(Wo[cols, :].astype(bf16_np)),
            "bq2": np.ascontiguousarray((bq[cols] * scale).reshape(NMC, 128).T),
            "bk2": np.ascontiguousarray(bk[cols].reshape(NMC, 128).T),
            "kbias": np.ascontiguousarray(kb_vec.reshape(NKC, 128).T),
        })

    from concourse.bass_utils import run_bass_kernel_spmd
    res = run_bass_kernel_spmd(nc, in_maps, list(range(N_CORES)))
    _LAST = res

    bo_eff = bo + bv @ Wo
    out = np.empty((B, S_q, E), np.float32)
    for b in range(B):
        out[b] = (res.results[2 * b]["oT"].astype(np.float32)
                  + res.results[2 * b + 1]["oT"].astype(np.float32)).T
        out[b] += bo_eff
    return out


# revision 31
# speedup vs baseline: 1.1183x; 1.1183x over previous
"""Multi-head attention (B=4, S=2048, E=768, H=12) on 8 trn2 NeuronCores.

Sharding: 2-D (batch x head-half). Core c handles batch c//2, heads
(c%2)*6 .. (c%2)*6+5  (Wq/Wk/Wv column-split, Wo row-split). Each core
returns a partial O^T [768, S]; host sums the two head-halves per batch,
transposes, and adds the effective output bias (bo + bv@Wo — softmax rows
sum to 1, so V's bias contributes a constant row folded on the host).

Device kernel (per core), bf16 matmuls + fp32 PSUM:
  - masked keys are compacted away on host; padded keys get -30000 added
    via the exp's per-partition bias -> exp == 0.
  - scores/ctx computed transposed (S^T tiles [128 k, q]) so P^T feeds the
    context matmul directly; V carries an appended ones column so row 64
    of the context accumulator is the softmax denominator.
  - the scalar (ACT) engine runs ONLY the exps; each head-pair's two score
    tiles live in one [128, 2, 512] PSUM tile so a single exp covers both.
  - normalization per (head, qblock): DMA the denominator row to
    partition 0, reciprocal there, exact hi/lo bf16 split, and broadcast
    across the 64 context rows with two accumulating K=1 ones-matmuls.
  - inputs stream in deadline order (K, Q-head, V-head, V-tail, Q-tail);
    only K/Q m=0 head-tiles project before attention — everything else
    (V, remaining projections, O-projection, normalization) is deferred
    into the attention stream in ~1us chunks popped once per key-chunk,
    so exps start ~15us in and the PE stays dense (HAM at full clock).
    Attention blocks run head-pair-outer so deferred m=1,2 projections
    have 4 blocks of slack before their consumers.
"""

import os
import numpy as np
import ml_dtypes

E = 768
H = 12
D = 64
HALF = 384  # E // 2 output cols per head-half
N_CORES = 8
QB = 512    # query block

_CACHE = {}
_LAST = None  # last BassKernelResults (for test harness introspection)

bf16_np = ml_dtypes.bfloat16


def _build(S_q, S_pad):
    from contextlib import ExitStack
    import concourse.bass as bass
    import concourse.tile as tile
    from concourse import bacc, mybir

    bf16 = mybir.dt.bfloat16
    f32 = mybir.dt.float32
    FT = mybir.ActivationFunctionType

    NKC = S_pad // 128
    NMC = HALF // 128        # 3 proj-dim chunks (head pairs)
    NEC = E // 128           # 6 embed chunks
    NQB = S_q // QB          # query blocks
    QA = 1024                # qT head split (first two n-tiles)
    VA = 512                 # vT head split (first four key chunks)

    def ntiles(total, step=512):
        return [(s, min(step, total - s)) for s in range(0, total, step)]

    nc = bacc.Bacc("TRN2", target_bir_lowering=False, debug=False,
                   num_devices=N_CORES)

    # inputs are interleaved host-side to [128, chunks, cols] so each
    # tensor loads with a single large DMA (trigger rate, not bandwidth,
    # limits the input stream otherwise)
    qTa = nc.dram_tensor("qTa", [128, NEC, QA], bf16, kind="ExternalInput").ap()
    qTb = nc.dram_tensor("qTb", [128, NEC, S_q - QA], bf16, kind="ExternalInput").ap()
    KA = 1024                # kT head split (first two n-tiles)
    kTa = nc.dram_tensor("kTa", [128, NEC, KA], bf16, kind="ExternalInput").ap()
    kTb = nc.dram_tensor("kTb", [128, NEC, S_pad - KA], bf16, kind="ExternalInput").ap()
    vTa = nc.dram_tensor("vTa", [128, NEC, VA], bf16, kind="ExternalInput").ap()
    vTb = nc.dram_tensor("vTb", [128, NEC, S_pad - VA], bf16, kind="ExternalInput").ap()
    wq = nc.dram_tensor("wq", [128, NEC, HALF], bf16, kind="ExternalInput").ap()
    wk = nc.dram_tensor("wk", [128, NEC, HALF], bf16, kind="ExternalInput").ap()
    wv = nc.dram_tensor("wv", [128, NEC, HALF], bf16, kind="ExternalInput").ap()
    wo = nc.dram_tensor("wo", [128, NMC, E], bf16, kind="ExternalInput").ap()
    bq2 = nc.dram_tensor("bq2", [128, NMC], f32, kind="ExternalInput").ap()
    bk2 = nc.dram_tensor("bk2", [128, NMC], f32, kind="ExternalInput").ap()
    kbias = nc.dram_tensor("kbias", [128, NKC], f32, kind="ExternalInput").ap()
    oT = nc.dram_tensor("oT", [E, S_q], bf16, kind="ExternalOutput").ap()

    with tile.TileContext(nc) as tc, ExitStack() as ctx:
        cons = ctx.enter_context(tc.tile_pool(name="cons", bufs=1))
        wp = ctx.enter_context(tc.tile_pool(name="wp", bufs=1))
        acts = ctx.enter_context(tc.tile_pool(name="acts", bufs=1))
        pp = ctx.enter_context(tc.tile_pool(name="pp", bufs=12))
        ost = ctx.enter_context(tc.tile_pool(name="ost", bufs=4))
        nrm = ctx.enter_context(tc.tile_pool(name="nrm", bufs=1))

        # ---- constant/small loads ----
        bq2_t = cons.tile([128, NMC], f32, tag="bq2")
        bk2_t = cons.tile([128, NMC], f32, tag="bk2")
        kb_t = cons.tile([128, NKC], f32, tag="kb")
        ones2 = cons.tile([2, 64], bf16, tag="ones2")
        nc.sync.dma_start(bq2_t[:], bq2[:])
        nc.sync.dma_start(bk2_t[:], bk2[:])
        nc.sync.dma_start(kb_t[:], kbias[:])
        nc.vector.memset(ones2[:], 1.0)

        # ---- input loads in deadline order, two trigger queues ----
        qkv = tc.tile_pool(name="qkv", bufs=1)
        inp = qkv.__enter__()
        wq_t = wp.tile([128, NEC, HALF], bf16, tag="wq", name="wq")
        wk_t = wp.tile([128, NEC, HALF], bf16, tag="wk", name="wk")
        wv_t = wp.tile([128, NEC, HALF], bf16, tag="wv", name="wv")
        wo_t = wp.tile([128, NMC, E], bf16, tag="wo", name="wo")
        kTa_t = inp.tile([128, NEC, KA], bf16, tag="kTa", name="kTat")
        kTb_t = inp.tile([128, NEC, S_pad - KA], bf16, tag="kTb", name="kTbt")
        qTa_t = inp.tile([128, NEC, QA], bf16, tag="qTa", name="qTat")
        qTb_t = inp.tile([128, NEC, S_q - QA], bf16, tag="qTb", name="qTbt")
        vTa_t = inp.tile([128, NEC, VA], bf16, tag="vTa", name="vTat")
        vTb_t = inp.tile([128, NEC, S_pad - VA], bf16, tag="vTb", name="vTbt")
        nc.sync.dma_start(wk_t[:], wk[:])
        nc.sync.dma_start(kTa_t[:], kTa[:])
        nc.sync.dma_start(wq_t[:], wq[:])
        nc.sync.dma_start(qTa_t[:], qTa[:])
        nc.sync.dma_start(wv_t[:], wv[:])
        nc.sync.dma_start(vTa_t[:], vTa[:])
        nc.sync.dma_start(vTb_t[:], vTb[:])
        nc.sync.dma_start(kTb_t[:], kTb[:])
        nc.sync.dma_start(qTb_t[:], qTb[:])
        nc.sync.dma_start(wo_t[:], wo[:])

        def wqx(e):
            return wq_t[:, e, :]

        def wkx(e):
            return wk_t[:, e, :]

        def qx(e, n0, nw):
            if n0 + nw <= QA:
                return qTa_t[:, e, n0:n0 + nw]
            return qTb_t[:, e, n0 - QA:n0 - QA + nw]

        def kx(e, n0, nw):
            if n0 + nw <= KA:
                return kTa_t[:, e, n0:n0 + nw]
            return kTb_t[:, e, n0 - KA:n0 - KA + nw]

        def vx(e, j):  # key chunk j of vT
            if 128 * (j + 1) <= VA:
                return vTa_t[:, e, 128 * j:128 * (j + 1)]
            return vTb_t[:, e, 128 * j - VA:128 * (j + 1) - VA]

        kts = [acts.tile([128, S_pad], bf16, tag=f"kts{m}", name=f"kts{m}") for m in range(NMC)]
        qts = [acts.tile([128, S_q], bf16, tag=f"qts{m}", name=f"qts{m}") for m in range(NMC)]
        vhx = [acts.tile([128, 6, 128], bf16, tag=f"vhx{j}", name=f"vhx{j}") for j in range(NKC)]
        czT = [acts.tile([128, S_q], bf16, tag=f"czT{m}", name=f"czT{m}") for m in range(NMC)]

        def proj_mm(ps_, tagf, wt, xf, m, pair, e_lo, e_hi, pjs):
            if not pjs:
                pjs.extend(ps_.tile([128, 512], f32, tag=tagf(i), bufs=2,
                                    name=f"pj{m}_{pair[i][0]}_{id(wt)}")
                           for i in range(len(pair)))
            for e in range(e_lo, e_hi):
                for i, (n0, nw) in enumerate(pair):
                    nc.tensor.matmul(pjs[i][:, :nw],
                                     wt[:, e, 128 * m:128 * (m + 1)],
                                     xf(e, n0, nw),
                                     start=(e == 0), stop=(e == NEC - 1))

        def proj_evac(out, bias_t, m, pair, pjs):
            for i, (n0, nw) in enumerate(pair):
                nc.vector.tensor_scalar_add(out[m][:, n0:n0 + nw],
                                            pjs[i][:, :nw],
                                            bias_t[:, m:m + 1])

        def v_mm(ps_, tagf, chunks, e_lo, e_hi, pvs):
            if not pvs:
                pvs.extend(ps_.tile([128, 512], f32, tag=tagf(i), bufs=2,
                                    name=f"pv{chunks[i]}")
                           for i in range(len(chunks)))
            for e in range(e_lo, e_hi):
                for i, j in enumerate(chunks):
                    nc.tensor.matmul(pvs[i][:, 0:HALF], vx(e, j),
                                     wv_t[:, e, :],
                                     start=(e == 0), stop=(e == NEC - 1))

        def v_evac(chunks, pvs):
            for i, j in enumerate(chunks):
                nc.vector.memset(vhx[j][:, :, 64:128], 1.0)
                nc.vector.tensor_copy(
                    vhx[j][:, :, 0:64],
                    pvs[i][:, 0:HALF].rearrange("p (h d) -> p h d", h=6))

        # ---- prologue: K m0 n0-1 then Q m0 n0-1 (from the early tiles) ----
        kt2 = ntiles(S_pad)
        qt2 = ntiles(S_q)
        psp = tc.tile_pool(name="psp", bufs=1, space="PSUM")
        ps = psp.__enter__()
        pjs = []
        proj_mm(ps, lambda i: f"pj{i}", wk_t, kx, 0, kt2[0:2], 0, NEC, pjs)
        proj_evac(kts, bk2_t, 0, kt2[0:2], pjs)
        pjs = []
        proj_mm(ps, lambda i: f"pj{2 + i}", wq_t, qx, 0, qt2[0:2], 0, NEC, pjs)
        proj_evac(qts, bq2_t, 0, qt2[0:2], pjs)
        psp.__exit__(None, None, None)

        # ---- attention ----
        psa = tc.tile_pool(name="psa", bufs=1, space="PSUM")
        ps = psa.__enter__()

        q_bulk = []     # deferred projections: each fn is atomic (opens
                        # and closes its accumulation), so prio pops can
                        # interleave between bulk fns safely
        q_prio = []     # normalization A/B-stages + O-projection
        pend_b = []     # normalization B-stages awaiting a 1-block lag
        hl_t = {}
        pushed_units = [0] * NQB

        def defer_proj(wt, xf, out, bias_t, m, pair):
            def fn():
                pjs = []
                proj_mm(ps, lambda i: "aux", wt, xf, m, pair, 0, NEC, pjs)
                proj_evac(out, bias_t, m, pair, pjs)
            q_bulk.append(fn)

        def defer_v(chunks):
            def fn():
                pvs = []
                v_mm(ps, lambda i: "aux", chunks, 0, NEC, pvs)
                v_evac(chunks, pvs)
            q_bulk.append(fn)

        # queue order == deadline order; see module docstring.
        for c0 in range(0, NKC, 2):
            defer_v(list(range(c0, min(c0 + 2, NKC))))
        if kt2[2:]:
            defer_proj(wk_t, kx, kts, bk2_t, 0, kt2[2:])
        if qt2[2:4]:
            defer_proj(wq_t, qx, qts, bq2_t, 0, qt2[2:4])
        for m in range(1, NMC):
            defer_proj(wk_t, kx, kts, bk2_t, m, kt2[0:2])
            if kt2[2:]:
                defer_proj(wk_t, kx, kts, bk2_t, m, kt2[2:])
            defer_proj(wq_t, qx, qts, bq2_t, m, qt2[0:2])
            if qt2[2:4]:
                defer_proj(wq_t, qx, qts, bq2_t, m, qt2[2:4])

        def make_unit_a(u, dn):
            def unit_a():
                # r = 1/den at partition 0, rounded to bf16 for broadcast.
                rq = nrm.tile([1, QB], f32, tag="rq", bufs=4, name=f"rq{u}")
                nc.vector.reciprocal_approx_fast(rq[:], dn[:])
                hl = nrm.tile([1, QB], bf16, tag="hl", bufs=4, name=f"hl{u}")
                nc.vector.tensor_copy(hl[:], rq[:])
                hl_t[u] = hl
            return unit_a

        def make_unit_b(u, cs):
            qb, h = divmod(u, 6)
            m, half = divmod(h, 2)

            def unit_b():
                hl = hl_t.pop(u)
                bcp = ps.tile([128, QB], f32, tag="aux", bufs=2,
                              name=f"bcp{u}")
                nc.tensor.matmul(bcp[0:64, :], ones2[0:1, :], hl[:],
                                 start=True, stop=True)
                nc.vector.tensor_mul(
                    czT[m][64 * half:64 * (half + 1),
                           qb * QB:(qb + 1) * QB],
                    cs[0:64, :], bcp[0:64, :])
            return unit_b

        def make_oproj(qb):
            fns = []
            for e0 in range(0, NEC, 2):
                def fn(e0=e0, qb=qb):
                    ecs = (e0, e0 + 1)
                    pos = [ps.tile([128, QB], f32, tag="aux", bufs=2,
                                   name=f"po{qb}_{ec}") for ec in ecs]
                    for mm in range(NMC):
                        for j, ec in enumerate(ecs):
                            nc.tensor.matmul(
                                pos[j][:],
                                wo_t[:, mm, 128 * ec:128 * (ec + 1)],
                                czT[mm][:, qb * QB:(qb + 1) * QB],
                                start=(mm == 0), stop=(mm == NMC - 1))
                    for j, ec in enumerate(ecs):
                        ot = ost.tile([128, QB], bf16, tag="ot",
                                      name=f"ot{qb}_{ec}")
                        nc.vector.tensor_copy(ot[:], pos[j][:])
                        nc.sync.dma_start(
                            oT[128 * ec:128 * (ec + 1),
                               qb * QB:(qb + 1) * QB], ot[:])
                fns.append(fn)
            return fns

        def evac(u, C):
            # context + denominator row out of PSUM; den row DMA'd to
            # partition 0 for the A-stage reciprocal.
            cs = nrm.tile([65, QB], f32, tag="cs", bufs=8, name=f"cs{u}")
            nc.vector.tensor_copy(cs[:], C[0:65, :])
            dn = nrm.tile([1, QB], f32, tag="dn", bufs=4, name=f"dn{u}")
            nc.sync.dma_start(dn[:], cs[64:65, :])
            q_prio.append(make_unit_a(u, dn))
            pend_b.append((u, make_unit_b(u, cs)))

        def flush_b():
            # push last block's B-stages (their hi/lo is ready by now) and
            # any O-projection whose 6 units are all queued.
            while pend_b:
                u, fn = pend_b.pop(0)
                q_prio.append(fn)
                qb = u // 6
                pushed_units[qb] += 1
                if pushed_units[qb] == 6:
                    q_prio.extend(make_oproj(qb))

        pend_cs = []  # [(u, C), ...] awaiting evacuation
        nblk = 0
        for p in range(NMC):      # head pair: hA=2p (rows 0-63), hB=2p+1
            for qb in range(NQB):
                q0 = qb * QB
                hA, hB = 2 * p, 2 * p + 1
                CA = ps.tile([128, QB], f32, tag="CA", name=f"CA{qb}_{p}")
                CB = ps.tile([128, QB], f32, tag="CB", name=f"CB{qb}_{p}")
                flush_b()
                for pc in pend_cs:
                    evac(*pc)
                pend_cs = []

                def sc_pair(kc, St):
                    nc.tensor.matmul(
                        St[:, 0, :], kts[p][0:64, 128 * kc:128 * (kc + 1)],
                        qts[p][0:64, q0:q0 + QB],
                        start=True, stop=True, tile_position=(0, 0))
                    nc.tensor.matmul(
                        St[:, 1, :], kts[p][64:128, 128 * kc:128 * (kc + 1)],
                        qts[p][64:128, q0:q0 + QB],
                        start=True, stop=True, tile_position=(64, 0))

                S = ps.tile([128, 2, QB], f32, tag="S", bufs=2,
                            name=f"S{qb}_{p}_0")
                sc_pair(0, S)
                last = nblk == NMC * NQB - 1
                for kc in range(NKC):
                    # block 0 pops bulk from kc0: the V-projection fns must
                    # all EMIT before their first in-block readers (vhx[j]
                    # is read by the kc=j context matmul).
                    if q_prio and kc >= 1:
                        q_prio.pop(0)()
                        if last and q_prio:
                            q_prio.pop(0)()
                    if q_bulk and (nblk == 0 or kc % 3 == 1) and (
                            kc >= 1 or nblk == 0):
                        q_bulk.pop(0)()
                    S2 = None
                    if kc + 1 < NKC:
                        S2 = ps.tile([128, 2, QB], f32, tag="S", bufs=2,
                                     name=f"S{qb}_{p}_{kc + 1}")
                        sc_pair(kc + 1, S2)
                    if kc == 5:
                        flush_b()
                    P = pp.tile([128, 2, QB], bf16, tag="P",
                                name=f"P{qb}_{p}_{kc}")
                    nc.scalar.activation(P[:], S[:], FT.Exp,
                                         bias=kb_t[:, kc:kc + 1], scale=1.0)
                    nc.tensor.matmul(CA[:], vhx[kc][:, hA, :], P[:, 0, :],
                                     start=(kc == 0), stop=(kc == NKC - 1))
                    nc.tensor.matmul(CB[:], vhx[kc][:, hB, :], P[:, 1, :],
                                     start=(kc == 0), stop=(kc == NKC - 1))
                    S = S2
                pend_cs = [(qb * 6 + hA, CA), (qb * 6 + hB, CB)]
                nblk += 1

        # flush: evacuate last heads, push their B-stages, drain the queue
        flush_b()
        for pc in pend_cs:
            evac(*pc)
        flush_b()
        while q_prio or q_bulk:
            (q_prio if q_prio else q_bulk).pop(0)()
        psa.__exit__(None, None, None)
        qkv.__exit__(None, None, None)

    nc.compile()
    return nc


def _numpy_fallback(q, k, v, mask, Wq, bq, Wk, bk, Wv, bv, Wo, bo):
    B, Sq, _ = q.shape
    qh = (q @ Wq + bq).reshape(B, Sq, H, D).transpose(0, 2, 1, 3)
    kh = (k @ Wk + bk).reshape(B, -1, H, D).transpose(0, 2, 1, 3)
    vh = (v @ Wv + bv).reshape(B, -1, H, D).transpose(0, 2, 1, 3)
    s = np.einsum("bhqd,bhkd->bhqk", qh, kh) / np.sqrt(np.float32(D))
    s = s + np.where(mask == 0, np.float32(-1e9), np.float32(0))[:, None, None, :]
    s = s - s.max(-1, keepdims=True)
    w = np.exp(s)
    w = w / w.sum(-1, keepdims=True)
    ctx = np.einsum("bhqk,bhkd->bqhd", w, vh).reshape(B, Sq, E)
    return (ctx @ Wo + bo).astype(np.float32)


def kernel(q, k, v, mask, Wq, bq, Wk, bk, Wv, bv, Wo, bo):
    global _LAST
    q = np.asarray(q, np.float32)
    k = np.asarray(k, np.float32)
    v = np.asarray(v, np.float32)
    mask = np.asarray(mask)
    Wq = np.asarray(Wq, np.float32)
    bq = np.asarray(bq, np.float32)
    Wk = np.asarray(Wk, np.float32)
    bk = np.asarray(bk, np.float32)
    Wv = np.asarray(Wv, np.float32)
    bv = np.asarray(bv, np.float32)
    Wo = np.asarray(Wo, np.float32)
    bo = np.asarray(bo, np.float32)

    B, S_q, _ = q.shape
    idxs = [np.flatnonzero(mask[b]) for b in range(B)]
    ns = [len(ix) for ix in idxs]
    if (min(ns) == 0 or B * 2 != N_CORES or S_q % QB != 0
            or S_q < 2048 or max(ns) <= 512):
        return _numpy_fallback(q, k, v, mask, Wq, bq, Wk, bk, Wv, bv, Wo, bo)

    S_pad = ((max(ns) + 127) // 128) * 128
    NKC = S_pad // 128
    NMC = HALF // 128

    key = (S_q, S_pad)
    if key not in _CACHE:
        _CACHE[key] = _build(S_q, S_pad)
    nc = _CACHE[key]

    scale = np.float32(1.0 / np.sqrt(D))

    def ileave(a):  # [n*128, cols] -> [128, n, cols]
        n = a.shape[0] // 128
        return np.ascontiguousarray(
            a.reshape(n, 128, a.shape[1]).transpose(1, 0, 2))

    in_maps = []
    for c in range(N_CORES):
        b, j = divmod(c, 2)
        cols = slice(j * HALF, (j + 1) * HALF)
        kc_ = np.zeros((S_pad, E), np.float32)
        kc_[:ns[b]] = k[b][idxs[b]]
        vc_ = np.zeros((S_pad, E), np.float32)
        vc_[:ns[b]] = v[b][idxs[b]]
        kb_vec = np.zeros(S_pad, np.float32)
        kb_vec[ns[b]:] = -30000.0
        qT_ = np.ascontiguousarray(q[b].T).astype(bf16_np)
        vT_ = np.ascontiguousarray(vc_.T).astype(bf16_np)
        in_maps.append({
            "qTa": ileave(qT_[:, 0:1024]),
            "qTb": ileave(qT_[:, 1024:]),
            "kTa": ileave(np.ascontiguousarray(kc_.T[:, 0:1024]).astype(bf16_np)),
            "kTb": ileave(np.ascontiguousarray(kc_.T[:, 1024:]).astype(bf16_np)),
            "vTa": ileave(vT_[:, 0:512]),
            "vTb": ileave(vT_[:, 512:]),
            "wq": ileave((Wq[:, cols] * scale).astype(bf16_np)),
            "wk": ileave(Wk[:, cols].astype(bf16_np)),
            "wv": ileave(Wv[:, cols].astype(bf16_np)),
            "wo": ileave(Wo[cols, :].astype(bf16_np)),
            "bq2": np.ascontiguousarray((bq[cols] * scale).reshape(NMC, 128).T),
            "bk2": np.ascontiguousarray(bk[cols].reshape(NMC, 128).T),
            "kbias": np.ascontiguousarray(kb_vec.reshape(NKC, 128).T),
        })

    from concourse.bass_utils import run_bass_kernel_spmd
    res = run_bass_kernel_spmd(nc, in_maps, list(range(N_CORES)))
    _LAST = res

    bo_eff = bo + bv @ Wo
    out = np.empty((B, S_q, E), np.float32)
    for b in range(B):
        out[b] = (res.results[2 * b]["oT"].astype(np.float32)
                  + res.results[2 * b + 1]["oT"].astype(np.float32)).T
        out[b] += bo_eff
    return out
